# revision 2
# baseline (speedup 1.0000x reference)
"""Edge-parallel GNN message-passing kernel for 8 Trainium2 NeuronCores.

v4: node features are SHARDED across the 8 cores (3.2MB each instead of a
15MB replicated gather table); the full table is assembled on-device with an
AllGather collective, and the per-edge source gathers read it through four
bucketed sub-gathers (int16 gather indices only span 25088 rows, so each
chunk's 2048 slots are laid out as four fixed 512-slot bucket runs).
Destination gathers read the core's own shard directly.  WINDOW=512 with one
2048-edge group per chunk; each chunk's gated messages are segment-summed
into a [128, 512] PSUM window and a dense [512, 128] staging tile is
written.

  * Host: sort edges by destination, split nodes into 8 contiguous ranges;
    each core gets every edge whose destination is in its range.
  * Node-level src/dst transforms folded into the first edge MLP layer on
    the host (linear, no activation in between):
      h1_pre = fsrc @ (W_src@W1a) + fdst @ (W_dst@W1b) + b1f
      g1_pre = fsrc @ (W_src@Wg1a) + fdst @ (W_dst@Wg1b) + bg1f
  * Device, per 2048-edge chunk: gather fp16 feature rows transposed
    ([feat, edge] layout) with dma_gather, run the MLP/gate/LayerNorm with
    fp32 PSUM accumulation, segment-sum into a 512-node window via one-hot
    matmuls, apply W_out, write a dense [512, 128] staging tile.
  * Host: accumulate the (overlapping) staging windows into [N, DOUT].
"""

import sys

sys.path.insert(0, "/opt/trn_rl_repo")

import numpy as np

import concourse.bass as bass
import concourse.bacc as bacc
import concourse.tile as tile
from concourse import mybir
from concourse.bass_utils import run_bass_kernel_spmd

N_CORES = 8
H = 128
WINDOW = 512          # destination-node window per chunk (PSUM free dim)
CHUNK_E = 2048        # edge slots per chunk
NSUB = CHUNK_E // 128  # 16 subtiles
NBUCK = 4             # gather-index buckets (int16 limit / table rows)
RUN = CHUNK_E // NBUCK  # slots per bucket run (fixed, SPMD-uniform)
LN_EPS = 1e-5
F16 = mybir.dt.float16
F32 = mybir.dt.float32
I16 = mybir.dt.int16
I32 = mybir.dt.int32
AF = mybir.ActivationFunctionType
ALU = mybir.AluOpType


# --------------------------------------------------------------------------
# host-side packing
# --------------------------------------------------------------------------

def _pack_core(ed_c, buck_c):
    """Split one core's dst-sorted edges into chunks whose destinations span
    <WINDOW nodes and which have <=RUN edges in each source bucket.
    Returns per-chunk (start, end, base)."""
    out = []
    i = 0
    n = len(ed_c)
    # prefix counts per bucket for O(1) range counts
    pref = np.zeros((NBUCK, n + 1), np.int64)
    for b in range(NBUCK):
        pref[b, 1:] = np.cumsum(buck_c == b)
    while i < n:
        base = int(ed_c[i])
        j = min(
            i + CHUNK_E,
            int(np.searchsorted(ed_c, base + WINDOW, side="left")),
        )
        # shrink j until every bucket count fits in RUN
        while True:
            over = 0
            for b in range(NBUCK):
                cnt = int(pref[b, j] - pref[b, i])
                if cnt > RUN:
                    over = max(over, cnt - RUN)
            if over == 0:
                break
            j -= over
        out.append((i, j, base))
        i = j
    return out


def _prepare(inputs):
    feat = np.ascontiguousarray(np.asarray(inputs["feat"], np.float32))
    es = np.asarray(inputs["edge_src"]).astype(np.int64)
    ed = np.asarray(inputs["edge_dst"]).astype(np.int64)
    N, DIN = feat.shape
    E = es.shape[0]
    npc = -(-N // N_CORES)  # nodes per core (ceil)

    f64 = np.float64
    W_src = np.asarray(inputs["W_src"], f64)
    W_dst = np.asarray(inputs["W_dst"], f64)
    W1a = np.asarray(inputs["W1a"], f64)
    W1b = np.asarray(inputs["W1b"], f64)
    Wg1a = np.asarray(inputs["Wg1a"], f64)
    Wg1b = np.asarray(inputs["Wg1b"], f64)
    b_src = np.asarray(inputs["b_src"], f64)
    b_dst = np.asarray(inputs["b_dst"], f64)
    ln_g = np.asarray(inputs["ln_g"], f64)
    ln_b = np.asarray(inputs["ln_b"], f64)
    if not np.allclose(ln_b, 0.0):
        raise NotImplementedError("non-zero ln_b not supported")

    wpack = {
        "A1s": W_src @ W1a,
        "A1d": W_dst @ W1b,
        "Ag1s": W_src @ Wg1a,
        "Ag1d": W_dst @ Wg1b,
        "W2": np.asarray(inputs["W2"], f64),
        "W3": np.asarray(inputs["W3"], f64),
        "W_out": np.diag(ln_g) @ np.asarray(inputs["W_out"], f64),
    }
    b1f = np.asarray(inputs["b1"], f64) + b_src @ W1a + b_dst @ W1b
    bg1f = np.asarray(inputs["bg1"], f64) + b_src @ Wg1a + b_dst @ Wg1b
    Wg2 = np.asarray(inputs["Wg2"], f64)  # [H, 1]
    bg2 = float(np.asarray(inputs["bg2"], f64).reshape(()))

    feat16 = feat.astype(np.float16)
    npc_pad = -(-npc // 128) * 128       # padded shard rows
    browse = 2 * npc_pad                 # rows per gather bucket (int16-safe)

    # global source node id -> (bucket, in-bucket table row)
    def node_to_row(nid):
        return (nid // npc) * npc_pad + (nid % npc)

    order = np.argsort(ed, kind="stable")
    es_s = es[order]
    ed_s = ed[order]
    src_rows = node_to_row(es_s)
    src_buck = src_rows // browse
    bounds = np.searchsorted(ed_s, np.arange(N_CORES + 1) * npc, side="left")

    core_chunks = []
    for c in range(N_CORES):
        lo, hi = int(bounds[c]), int(bounds[c + 1])
        core_chunks.append(_pack_core(ed_s[lo:hi], src_buck[lo:hi]))

    nchunk = max(len(g) for g in core_chunks)

    # device input arrays per core
    in_maps = []
    meta = []
    for c in range(N_CORES):
        lo = int(bounds[c])
        chunks = core_chunks[c]
        sidx = np.zeros((nchunk, CHUNK_E), np.int64)
        didx = np.zeros((nchunk, CHUNK_E), np.int64)
        lidx = np.full((nchunk, CHUNK_E), -1.0, np.float32)
        bases = np.zeros(nchunk, np.int64)
        c0 = c * npc
        for g, (i, j, base) in enumerate(chunks):
            bases[g] = base
            eb = src_buck[lo + i: lo + j]
            erow = src_rows[lo + i: lo + j] % browse
            edst = ed_s[lo + i: lo + j] - base
            eloc = ed_s[lo + i: lo + j] - c0
            for b in range(NBUCK):
                m = eb == b
                k = int(m.sum())
                assert k <= RUN
                s0 = b * RUN
                sidx[g, s0: s0 + k] = erow[m]
                didx[g, s0: s0 + k] = eloc[m]
                lidx[g, s0: s0 + k] = edst[m].astype(np.float32)

        fshard = np.zeros((npc_pad, H), np.float16)
        seg = feat16[c0: min(c0 + npc, N)]
        fshard[: len(seg)] = seg

        def wrap16(idx):  # [nchunk, CHUNK_E] -> [nchunk, 16, CHUNK_E//16]
            return np.ascontiguousarray(
                idx.reshape(nchunk, CHUNK_E // 16, 16).transpose(0, 2, 1)
            ).astype(np.int16)

        # lidx transposed for per-partition access: [128, NSUB*nchunk]
        lidxT = np.ascontiguousarray(
            lidx.reshape(nchunk, NSUB, 128).transpose(2, 0, 1).reshape(128, -1)
        ).astype(np.float32)

        im = {
            "fshard": fshard,
            "sidxw": wrap16(sidx),
            "didxw": wrap16(didx),
            "lidxT": lidxT,
            "b3rep": np.tile(np.asarray(inputs["b3"], np.float32), (128, 4)),
            "iota": np.tile(np.arange(WINDOW, dtype=np.float16), (128, 1)),
            "b1f": b1f.astype(np.float32).reshape(H, 1),
            "bg1f": bg1f.astype(np.float32).reshape(H, 1),
            "b2": np.asarray(inputs["b2"], np.float32).reshape(H, 1),
            "bg2": np.full((128, 1), 0.5 * bg2, np.float32),  # tanh-form sigmoid
            "Wg2": Wg2.astype(np.float16),
        }
        for k, v in wpack.items():
            im[k] = v.astype(np.float16)
        in_maps.append(im)
        meta.append(bases)

    b_out = np.asarray(inputs["b_out"], np.float64)
    return dict(
        in_maps=in_maps, meta=meta, nchunk=nchunk, npc_pad=npc_pad,
        browse=browse, N=N, b_out=b_out,
    )


# --------------------------------------------------------------------------
# device kernel builder
# --------------------------------------------------------------------------

def _build(nchunk, npc_pad, browse, reps=1):
    nc = bacc.Bacc("TRN2", target_bir_lowering=False, debug=False,
                   num_devices=N_CORES)
    d = {}
    d["fshard"] = nc.dram_tensor("fshard", [npc_pad, H], F16,
                                 kind="ExternalInput")
    d["sidxw"] = nc.dram_tensor("sidxw", [nchunk, 16, CHUNK_E // 16], I16,
                                kind="ExternalInput")
    d["didxw"] = nc.dram_tensor("didxw", [nchunk, 16, CHUNK_E // 16], I16,
                                kind="ExternalInput")
    d["lidxT"] = nc.dram_tensor("lidxT", [128, NSUB * nchunk], F32,
                                kind="ExternalInput")
    d["b3rep"] = nc.dram_tensor("b3rep", [128, 4 * 128], F32,
                                kind="ExternalInput")
    d["iota"] = nc.dram_tensor("iota", [128, WINDOW], F16, kind="ExternalInput")
    for nm in ("b1f", "bg1f", "b2", "bg2"):
        d[nm] = nc.dram_tensor(nm, [128, 1], F32, kind="ExternalInput")
    for nm in ("A1s", "A1d", "Ag1s", "Ag1d", "W2", "W3", "W_out"):
        d[nm] = nc.dram_tensor(nm, [H, H], F16, kind="ExternalInput")
    d["Wg2"] = nc.dram_tensor("Wg2", [H, 1], F16, kind="ExternalInput")
    staging = nc.dram_tensor("staging", [nchunk, 4, 128, 128], F16,
                             kind="ExternalOutput")

    with tile.TileContext(nc) as tc:
        with (
            tc.tile_pool(name="dram", bufs=1, space="DRAM") as dram,
            tc.tile_pool(name="singles", bufs=1) as singles,
            tc.tile_pool(name="gath", bufs=2) as gath,
            tc.tile_pool(name="acts", bufs=3) as acts,
            tc.tile_pool(name="ln", bufs=3) as lnp,
            tc.tile_pool(name="outp", bufs=3) as outp,
            tc.tile_pool(name="ppack", bufs=3, space="PSUM") as ppack,
            tc.tile_pool(name="pwin", bufs=2, space="PSUM") as pwin,
            tc.tile_pool(name="psmall", bufs=2, space="PSUM") as psmall,
        ):
            # ---- preamble: AllGather the feature shards into a full
            #      on-device table (collectives cannot read IO tensors,
            #      so bounce through an internal DRAM tile) ----
            bounce = dram.tile([npc_pad, H], F16, tag="bounce")
            nc.sync.dma_start(out=bounce[:], in_=d["fshard"][:, :])
            ftab = dram.tile([N_CORES, npc_pad, H], F16, tag="ftab")
            import os as _os
            if int(_os.environ.get("KERNEL_NO_COLL", "0")):
                nc.sync.dma_start(out=ftab[0], in_=bounce[:])
            else:
                nc.gpsimd.collective_compute(
                    "AllGather",
                    mybir.AluOpType.bypass,
                    replica_groups=[list(range(N_CORES))],
                    ins=[bounce[:]],
                    outs=[ftab[:]],
                )
            ftab_rows = ftab[:].rearrange("c r f -> (c r) f")

            # ---- preamble: constants into SBUF ----
            w = {}
            for nm in ("A1s", "A1d", "Ag1s", "Ag1d", "W2", "W3", "W_out"):
                w[nm] = singles.tile([H, H], F16, tag=nm, name=nm)
                nc.sync.dma_start(out=w[nm], in_=d[nm][:, :])
            w["Wg2"] = singles.tile([H, 1], F16, tag="Wg2", name="Wg2")
            nc.sync.dma_start(out=w["Wg2"], in_=d["Wg2"][:, :])
            bias = {}
            for nm in ("b1f", "bg1f", "b2", "bg2"):
                bias[nm] = singles.tile([128, 1], F32, tag=nm, name=nm)
                nc.sync.dma_start(out=bias[nm], in_=d[nm][:, :])
            b3rep = singles.tile([128, 4 * 128], F32, tag="b3rep")
            nc.sync.dma_start(out=b3rep, in_=d["b3rep"][:, :])
            iota = singles.tile([128, WINDOW], F16, tag="iota")
            nc.sync.dma_start(out=iota, in_=d["iota"][:, :])
            lidxT = singles.tile([128, NSUB * nchunk], F32, tag="lidxT")
            nc.sync.dma_start(out=lidxT, in_=d["lidxT"][:, :])
            sidx_sb = singles.tile([128, nchunk, CHUNK_E // 16], I16, tag="sidx")
            didx_sb = singles.tile([128, nchunk, CHUNK_E // 16], I16, tag="didx")
            for r in range(8):
                nc.sync.dma_start(
                    out=sidx_sb[r * 16:(r + 1) * 16],
                    in_=d["sidxw"].rearrange("c p e -> p c e"),
                )
                nc.sync.dma_start(
                    out=didx_sb[r * 16:(r + 1) * 16],
                    in_=d["didxw"].rearrange("c p e -> p c e"),
                )
            # bias tile for Rsqrt(4*(var+eps)) = 0.5/sqrt(var+eps)
            eps4 = singles.tile([128, 1], F32, tag="eps4")
            nc.vector.memset(eps4, 4.0 * LN_EPS)

            import os as _os2
            skip_compute = int(_os2.environ.get("KERNEL_SKIP_COMPUTE", "0"))
            skip_gather = int(_os2.environ.get("KERNEL_SKIP_GATHER", "0"))
            no_act = int(_os2.environ.get("KERNEL_NO_ACT", "0"))
            no_bn = int(_os2.environ.get("KERNEL_NO_BN", "0"))
            no_ts = int(_os2.environ.get("KERNEL_NO_TS", "0"))
            AF_G = AF.Copy if no_act else AF.Gelu
            AF_T = AF.Copy if no_act else AF.Tanh
            AF_S = AF.Copy if no_act else AF.Sqrt
            for _rep in range(reps):
              for c in range(nchunk):
                fsT = gath.tile([128, 1, CHUNK_E], F16, tag="fsT")
                for b in range(NBUCK) if not skip_gather else []:
                    nc.gpsimd.dma_gather(
                        out_ap=fsT[:, :, b * RUN:(b + 1) * RUN],
                        in_ap=ftab_rows[b * browse:(b + 1) * browse, :],
                        idxs_ap=sidx_sb[:, c, b * (RUN // 16):
                                        (b + 1) * (RUN // 16)],
                        num_idxs=RUN,
                        num_idxs_reg=RUN,
                        elem_size=H,
                        transpose=True,
                        single_packet=False,
                        queue_num=0,
                    )
                fdT = gath.tile([128, 1, CHUNK_E], F16, tag="fdT")
                if skip_gather:
                    nc.vector.memset(fdT[:, :, 0:2], 0)
                    nc.vector.memset(fsT[:, :, 0:2], 0)
                else:
                  nc.gpsimd.dma_gather(
                    out_ap=fdT,
                    in_ap=d["fshard"][:, :],
                    idxs_ap=didx_sb[:, c, :],
                    num_idxs=CHUNK_E,
                    num_idxs_reg=CHUNK_E,
                    elem_size=H,
                    transpose=True,
                    single_packet=False,
                    queue_num=0,
                  )
                if skip_compute:
                    z16 = outp.tile([128, 4, 128], F16, tag="osb")
                    nc.vector.tensor_copy(z16, fsT.rearrange(
                        "p a (q f) -> p (a q) f", q=4)[:, :, 0:128])
                    nc.sync.dma_start(
                        out=staging[c].rearrange("hh j d -> j hh d"),
                        in_=z16,
                    )
                    continue
                # -- phase A: edge MLP + LN stats per 512-quarter; gate
                #    matmuls accumulate into one chunk-wide PSUM tile --
                gatep = psmall.tile([128, NSUB], F32, tag="gatep", bufs=2)
                mv = lnp.tile([128, NSUB, 2], F32, tag="mv")
                xs = []
                for gi in range(4):
                    e0 = gi * 512
                    fs = fsT[:, 0, e0:e0 + 512]
                    fd = fdT[:, 0, e0:e0 + 512]

                    h1p = ppack.tile([128, 512], F32, tag="big")
                    nc.tensor.matmul(h1p, w["A1s"], fs, start=True, stop=False)
                    nc.tensor.matmul(h1p, w["A1d"], fd, start=False, stop=True)
                    g1p = ppack.tile([128, 512], F32, tag="big")
                    nc.tensor.matmul(g1p, w["Ag1s"], fs, start=True, stop=False)
                    nc.tensor.matmul(g1p, w["Ag1d"], fd, start=False, stop=True)

                    h1s = acts.tile([128, 512], F16, tag="h1s")
                    nc.scalar.activation(h1s, h1p, AF_G, bias=bias["b1f"])
                    h2p = ppack.tile([128, 512], F32, tag="big")
                    nc.tensor.matmul(h2p, w["W2"], h1s, start=True, stop=True)
                    h2s = acts.tile([128, 512], F16, tag="h2s")
                    nc.scalar.activation(h2s, h2p, AF_G, bias=bias["b2"])
                    g1s = acts.tile([128, 512], F16, tag="g1s")
                    nc.scalar.activation(g1s, g1p, AF_G, bias=bias["bg1f"])

                    # msg_pre (un-transposed, [edge, feat]) and gate pre-act
                    msgp = ppack.tile([128, 512], F32, tag="big")
                    for s in range(4):
                        sl = slice(s * 128, (s + 1) * 128)
                        nc.tensor.matmul(
                            msgp[:, sl], h2s[:, sl], w["W3"],
                            start=True, stop=True, skip_group_check=True,
                        )
                        k = gi * 4 + s
                        nc.tensor.matmul(
                            gatep[:, k:k + 1], g1s[:, sl], w["Wg2"],
                            start=True, stop=True, skip_group_check=True,
                        )
                    # x = msg_pre + b3 (fp16), then per-subtile LN stats
                    x = lnp.tile([128, 4, 128], F16, tag="x", bufs=8)
                    nc.vector.tensor_tensor(
                        x, msgp.rearrange("p (s f) -> p s f", s=4),
                        b3rep.rearrange("p (s f) -> p s f", s=4),
                        op=ALU.add,
                    )
                    xs.append(x)
                    if no_bn:
                        nc.vector.memset(mv[:, gi * 4:(gi + 1) * 4, :], 0.25)
                    else:
                        st = lnp.tile([128, 4, 6], F32, tag="st")
                        for s in range(4):
                            k = gi * 4 + s
                            nc.vector.bn_stats(st[:, s, :], x[:, s, :])
                            nc.vector.bn_aggr(mv[:, k, :], st[:, s, :])

                # -- phase B (chunk-wide): gate = 0.5*(tanh+1) and
                #    0.5*rstd = Rsqrt(4*(var+eps)); sc = their product.
                #    Scalar/vector engines only — gpsimd is reserved for
                #    dma_gather descriptor generation --
                NG = NSUB
                gate_t = lnp.tile([128, NG], F32, tag="gate_t")
                nc.scalar.activation(gate_t, gatep, AF_T,
                                     bias=bias["bg2"], scale=0.5)
                g2t = lnp.tile([128, NG], F32, tag="g2t")
                nc.vector.tensor_scalar(g2t, gate_t, 1.0, None, op0=ALU.add)
                std2 = lnp.tile([128, NG], F32, tag="std2")
                nc.scalar.activation(std2, mv[:, :, 1], AF_S,
                                     bias=eps4, scale=4.0)
                rstd_h = lnp.tile([128, NG], F32, tag="rstd_h")
                nc.vector.reciprocal(rstd_h, std2)
                sc = lnp.tile([128, NG], F32, tag="sc")
                nc.vector.tensor_tensor(sc, rstd_h, g2t, op=ALU.mult)

                # -- phase C: center, one-hot (scaled), segment-sum into the
                #    chunk-wide 512 window, then W_out --
                updp = pwin.tile([128, WINDOW], F32, tag="win")
                for gi in range(4):
                    x = xs[gi]
                    msg16 = acts.tile([128, 512], F16, tag="msg16")
                    A = acts.tile([128, 4, WINDOW], F16, tag="A", bufs=4)
                    if no_ts:
                        nc.vector.tensor_copy(msg16, x.rearrange("p s f -> p (s f)"))
                        nc.vector.memset(A, 0.001)
                    else:
                      for s in range(4):
                        sl = slice(s * 128, (s + 1) * 128)
                        k = gi * 4 + s
                        nc.vector.tensor_scalar(
                            msg16[:, sl], x[:, s, :],
                            mv[:, k, 0:1], None, op0=ALU.subtract,
                        )
                        nc.vector.tensor_scalar(
                            A[:, s, :], iota,
                            lidxT[:, c * NSUB + k: c * NSUB + k + 1],
                            sc[:, k:k + 1],
                            op0=ALU.is_equal, op1=ALU.mult,
                        )
                    for s in range(4):
                        sl = slice(s * 128, (s + 1) * 128)
                        k = gi * 4 + s
                        nc.tensor.matmul(
                            updp, msg16[:, sl], A[:, s, :],
                            start=(k == 0), stop=(k == NSUB - 1),
                            skip_group_check=True,
                        )
                upd16 = outp.tile([128, WINDOW], F16, tag="upd16")
                if c % 2 == 0:
                    nc.vector.tensor_copy(upd16, updp)
                else:
                    nc.scalar.activation(upd16, updp, AF.Copy)

                o4 = psmall.tile([128, 4, 128], F32, tag="o4", bufs=1)
                for hh in range(4):
                    nc.tensor.matmul(
                        o4[:, hh, :], upd16[:, hh * 128:(hh + 1) * 128],
                        w["W_out"], start=True, stop=True,
                        skip_group_check=True,
                    )
                osb = outp.tile([128, 4, 128], F16, tag="osb")
                if c % 2 == 0:
                    nc.scalar.activation(osb, o4, AF.Copy)
                else:
                    nc.vector.tensor_copy(osb, o4)
                nc.sync.dma_start(
                    out=staging[c].rearrange("hh j d -> j hh d"),
                    in_=osb,
                )
    nc.finalize()
    return nc


# --------------------------------------------------------------------------
# entry point
# --------------------------------------------------------------------------

_LAST_PERF = {}


def kernel(**inputs):
    import os
    import time as _time
    prep = _prepare(inputs)
    reps = int(os.environ.get("KERNEL_REPS", "1"))
    nc = _build(prep["nchunk"], prep["npc_pad"], prep["browse"], reps=reps)
    trace = bool(int(os.environ.get("KERNEL_TRACE", "0")))
    res = run_bass_kernel_spmd(
        nc, prep["in_maps"], core_ids=list(range(N_CORES)), trace=trace,
    )
    nrep = int(os.environ.get("KERNEL_REPEAT", "0"))
    if nrep:
        walls = []
        for _ in range(nrep):
            t0 = _time.time()
            res = run_bass_kernel_spmd(
                nc, prep["in_maps"], core_ids=list(range(N_CORES)), trace=trace,
            )
            walls.append(_time.time() - t0)
        _rw = min(walls)
        print("repeat walls (ms):", " ".join("%.0f" % (w * 1e3) for w in walls))
    else:
        _rw = None
    _LAST_PERF.clear()
    _LAST_PERF.update(
        repeat_wall_s=_rw,
        exec_time_ns=res.exec_time_ns,
        mean_exec_time_ns=res.mean_exec_time_ns,
        trace=res.instructions_and_trace[1] if res.instructions_and_trace else None,
    )

    N = prep["N"]
    out = np.zeros((N + WINDOW, H), np.float64)
    for c in range(N_CORES):
        stg = res.results[c]["staging"].reshape(prep["nchunk"], WINDOW, H)
        bases = prep["meta"][c]
        for g in range(prep["nchunk"]):
            b = int(bases[g])
            out[b: b + WINDOW] += stg[g]
    out = out[:N] + prep["b_out"]
    return out.astype(np.float32)


# revision 6
# speedup vs baseline: 1.2406x; 1.2406x over previous
"""Edge-parallel GNN message-passing kernel for 8 Trainium2 NeuronCores.

v4: node features are SHARDED across the 8 cores (3.2MB each instead of a
15MB replicated gather table); the full table is assembled on-device with an
AllGather collective, and the per-edge source gathers read it through four
bucketed sub-gathers (int16 gather indices only span 25088 rows, so each
chunk's 2048 slots are laid out as four fixed 512-slot bucket runs).
Destination gathers read the core's own shard directly.  WINDOW=512 with one
2048-edge group per chunk; each chunk's gated messages are segment-summed
into a [128, 512] PSUM window and a dense [512, 128] staging tile is
written.

  * Host: sort edges by destination, split nodes into 8 contiguous ranges;
    each core gets every edge whose destination is in its range.
  * Node-level src/dst transforms folded into the first edge MLP layer on
    the host (linear, no activation in between):
      h1_pre = fsrc @ (W_src@W1a) + fdst @ (W_dst@W1b) + b1f
      g1_pre = fsrc @ (W_src@Wg1a) + fdst @ (W_dst@Wg1b) + bg1f
  * Device, per 2048-edge chunk: gather fp16 feature rows transposed
    ([feat, edge] layout) with dma_gather, run the MLP/gate/LayerNorm with
    fp32 PSUM accumulation, segment-sum into a 512-node window via one-hot
    matmuls, apply W_out, write a dense [512, 128] staging tile.
  * Host: accumulate the (overlapping) staging windows into [N, DOUT].
"""

import sys

sys.path.insert(0, "/opt/trn_rl_repo")

import numpy as np

import concourse.bass as bass
import concourse.bacc as bacc
import concourse.tile as tile
from concourse import mybir
from concourse.bass_utils import run_bass_kernel_spmd

N_CORES = 8
H = 128
WINDOW = 512          # destination-node window per chunk (PSUM free dim)
CHUNK_E = 2048        # edge slots per chunk
NSUB = CHUNK_E // 128  # 16 subtiles
NBUCK = 4             # gather-index buckets (int16 limit / table rows)
RUN = CHUNK_E // NBUCK  # slots per bucket run (fixed, SPMD-uniform)
LN_EPS = 1e-5
F16 = mybir.dt.float16
F32 = mybir.dt.float32
I16 = mybir.dt.int16
I32 = mybir.dt.int32
AF = mybir.ActivationFunctionType
ALU = mybir.AluOpType


# --------------------------------------------------------------------------
# host-side packing
# --------------------------------------------------------------------------

def _pack_core(ed_c, buck_c):
    """Split one core's dst-sorted edges into chunks whose destinations span
    <WINDOW nodes and which have <=RUN edges in each source bucket.
    Returns per-chunk (start, end, base)."""
    out = []
    i = 0
    n = len(ed_c)
    # prefix counts per bucket for O(1) range counts
    pref = np.zeros((NBUCK, n + 1), np.int64)
    for b in range(NBUCK):
        pref[b, 1:] = np.cumsum(buck_c == b)
    while i < n:
        base = int(ed_c[i])
        j = min(
            i + CHUNK_E,
            int(np.searchsorted(ed_c, base + WINDOW, side="left")),
        )
        # shrink j until every bucket count fits in RUN
        while True:
            over = 0
            for b in range(NBUCK):
                cnt = int(pref[b, j] - pref[b, i])
                if cnt > RUN:
                    over = max(over, cnt - RUN)
            if over == 0:
                break
            j -= over
        out.append((i, j, base))
        i = j
    return out


def _prepare(inputs):
    feat = np.ascontiguousarray(np.asarray(inputs["feat"], np.float32))
    es = np.asarray(inputs["edge_src"]).astype(np.int64)
    ed = np.asarray(inputs["edge_dst"]).astype(np.int64)
    N, DIN = feat.shape
    E = es.shape[0]
    npc = -(-N // N_CORES)  # nodes per core (ceil)

    f64 = np.float64
    W_src = np.asarray(inputs["W_src"], f64)
    W_dst = np.asarray(inputs["W_dst"], f64)
    W1a = np.asarray(inputs["W1a"], f64)
    W1b = np.asarray(inputs["W1b"], f64)
    Wg1a = np.asarray(inputs["Wg1a"], f64)
    Wg1b = np.asarray(inputs["Wg1b"], f64)
    b_src = np.asarray(inputs["b_src"], f64)
    b_dst = np.asarray(inputs["b_dst"], f64)
    ln_g = np.asarray(inputs["ln_g"], f64)
    ln_b = np.asarray(inputs["ln_b"], f64)
    if not np.allclose(ln_b, 0.0):
        raise NotImplementedError("non-zero ln_b not supported")

    wpack = {
        "A1s": W_src @ W1a,
        "A1d": W_dst @ W1b,
        "Ag1s": W_src @ Wg1a,
        "Ag1d": W_dst @ Wg1b,
        "W2": np.asarray(inputs["W2"], f64),
        "W3": np.asarray(inputs["W3"], f64),
        "W_out": np.diag(ln_g) @ np.asarray(inputs["W_out"], f64),
    }
    b1f = np.asarray(inputs["b1"], f64) + b_src @ W1a + b_dst @ W1b
    bg1f = np.asarray(inputs["bg1"], f64) + b_src @ Wg1a + b_dst @ Wg1b
    Wg2 = np.asarray(inputs["Wg2"], f64)  # [H, 1]
    bg2 = float(np.asarray(inputs["bg2"], f64).reshape(()))

    feat16 = feat.astype(np.float16)
    npc_pad = -(-npc // 128) * 128       # padded shard rows
    browse = 2 * npc_pad                 # rows per gather bucket (int16-safe)

    # global source node id -> (bucket, in-bucket table row)
    def node_to_row(nid):
        return (nid // npc) * npc_pad + (nid % npc)

    order = np.argsort(ed, kind="stable")
    es_s = es[order]
    ed_s = ed[order]
    src_rows = node_to_row(es_s)
    src_buck = src_rows // browse
    bounds = np.searchsorted(ed_s, np.arange(N_CORES + 1) * npc, side="left")

    core_chunks = []
    for c in range(N_CORES):
        lo, hi = int(bounds[c]), int(bounds[c + 1])
        core_chunks.append(_pack_core(ed_s[lo:hi], src_buck[lo:hi]))

    nchunk = max(len(g) for g in core_chunks)

    # device input arrays per core
    in_maps = []
    meta = []
    for c in range(N_CORES):
        lo = int(bounds[c])
        chunks = core_chunks[c]
        sidx = np.zeros((nchunk, CHUNK_E), np.int64)
        didx = np.zeros((nchunk, CHUNK_E), np.int64)
        lidx = np.full((nchunk, CHUNK_E), -1.0, np.float32)
        bases = np.zeros(nchunk, np.int64)
        c0 = c * npc
        for g, (i, j, base) in enumerate(chunks):
            bases[g] = base
            eb = src_buck[lo + i: lo + j]
            erow = src_rows[lo + i: lo + j] % browse
            edst = ed_s[lo + i: lo + j] - base
            eloc = ed_s[lo + i: lo + j] - c0
            for b in range(NBUCK):
                m = eb == b
                k = int(m.sum())
                assert k <= RUN
                s0 = b * RUN
                sidx[g, s0: s0 + k] = erow[m]
                didx[g, s0: s0 + k] = eloc[m]
                lidx[g, s0: s0 + k] = edst[m].astype(np.float32)

        fshard = np.zeros((npc_pad, H), np.float16)
        seg = feat16[c0: min(c0 + npc, N)]
        fshard[: len(seg)] = seg

        def wrap16(idx):  # [nchunk, CHUNK_E] -> [nchunk, 16, CHUNK_E//16]
            return np.ascontiguousarray(
                idx.reshape(nchunk, CHUNK_E // 16, 16).transpose(0, 2, 1)
            ).astype(np.int16)

        # lidx transposed for per-partition access: [128, NSUB*nchunk]
        lidxT = np.ascontiguousarray(
            lidx.reshape(nchunk, NSUB, 128).transpose(2, 0, 1).reshape(128, -1)
        ).astype(np.float32)

        im = {
            "fshard": fshard,
            "sidxw": wrap16(sidx),
            "didxw": wrap16(didx),
            "lidxT": lidxT,
            "b3rep": np.tile(np.asarray(inputs["b3"], np.float32), (128, 4)),
            "iota": np.tile(np.arange(WINDOW, dtype=np.float16), (128, 1)),
            "b1f": b1f.astype(np.float32).reshape(H, 1),
            "bg1f": bg1f.astype(np.float32).reshape(H, 1),
            "b2": np.asarray(inputs["b2"], np.float32).reshape(H, 1),
            "bg2": np.full((128, 1), 0.5 * bg2, np.float32),  # tanh-form sigmoid
            "Wg2": Wg2.astype(np.float16),
        }
        for k, v in wpack.items():
            im[k] = v.astype(np.float16)
        in_maps.append(im)
        meta.append(bases)

    b_out = np.asarray(inputs["b_out"], np.float64)
    return dict(
        in_maps=in_maps, meta=meta, nchunk=nchunk, npc_pad=npc_pad,
        browse=browse, N=N, b_out=b_out,
    )


# --------------------------------------------------------------------------
# device kernel builder
# --------------------------------------------------------------------------

def _build(nchunk, npc_pad, browse, reps=1):
    nc = bacc.Bacc("TRN2", target_bir_lowering=False, debug=False,
                   num_devices=N_CORES)
    d = {}
    d["fshard"] = nc.dram_tensor("fshard", [npc_pad, H], F16,
                                 kind="ExternalInput")
    d["sidxw"] = nc.dram_tensor("sidxw", [nchunk, 16, CHUNK_E // 16], I16,
                                kind="ExternalInput")
    d["didxw"] = nc.dram_tensor("didxw", [nchunk, 16, CHUNK_E // 16], I16,
                                kind="ExternalInput")
    d["lidxT"] = nc.dram_tensor("lidxT", [128, NSUB * nchunk], F32,
                                kind="ExternalInput")
    d["b3rep"] = nc.dram_tensor("b3rep", [128, 4 * 128], F32,
                                kind="ExternalInput")
    d["iota"] = nc.dram_tensor("iota", [128, WINDOW], F16, kind="ExternalInput")
    for nm in ("b1f", "bg1f", "b2", "bg2"):
        d[nm] = nc.dram_tensor(nm, [128, 1], F32, kind="ExternalInput")
    for nm in ("A1s", "A1d", "Ag1s", "Ag1d", "W2", "W3", "W_out"):
        d[nm] = nc.dram_tensor(nm, [H, H], F16, kind="ExternalInput")
    d["Wg2"] = nc.dram_tensor("Wg2", [H, 1], F16, kind="ExternalInput")
    staging = nc.dram_tensor("staging", [nchunk, 4, 128, 128], F16,
                             kind="ExternalOutput")

    with tile.TileContext(nc) as tc:
        with (
            tc.tile_pool(name="dram", bufs=1, space="DRAM") as dram,
            tc.tile_pool(name="singles", bufs=1) as singles,
            tc.tile_pool(name="gath", bufs=2) as gath,
            tc.tile_pool(name="acts", bufs=3) as acts,
            tc.tile_pool(name="ln", bufs=3) as lnp,
            tc.tile_pool(name="outp", bufs=3) as outp,
            tc.tile_pool(name="ppack", bufs=3, space="PSUM") as ppack,
            tc.tile_pool(name="pwin", bufs=2, space="PSUM") as pwin,
            tc.tile_pool(name="psmall", bufs=2, space="PSUM") as psmall,
        ):
            # ---- preamble: AllGather the feature shards into a full
            #      on-device table (collectives cannot read IO tensors,
            #      so bounce through an internal DRAM tile) ----
            bounce = dram.tile([npc_pad, H], F16, tag="bounce")
            nc.sync.dma_start(out=bounce[:], in_=d["fshard"][:, :])
            ftab = dram.tile([N_CORES, npc_pad, H], F16, tag="ftab")
            import os as _os
            if int(_os.environ.get("KERNEL_NO_COLL", "0")):
                nc.sync.dma_start(out=ftab[0], in_=bounce[:])
            else:
                nc.gpsimd.collective_compute(
                    "AllGather",
                    mybir.AluOpType.bypass,
                    replica_groups=[list(range(N_CORES))],
                    ins=[bounce[:]],
                    outs=[ftab[:]],
                )
            ftab_rows = ftab[:].rearrange("c r f -> (c r) f")

            # ---- preamble: constants into SBUF ----
            w = {}
            for nm in ("A1s", "A1d", "Ag1s", "Ag1d", "W2", "W3", "W_out"):
                w[nm] = singles.tile([H, H], F16, tag=nm, name=nm)
                nc.sync.dma_start(out=w[nm], in_=d[nm][:, :])
            w["Wg2"] = singles.tile([H, 1], F16, tag="Wg2", name="Wg2")
            nc.sync.dma_start(out=w["Wg2"], in_=d["Wg2"][:, :])
            bias = {}
            for nm in ("b1f", "bg1f", "b2", "bg2"):
                bias[nm] = singles.tile([128, 1], F32, tag=nm, name=nm)
                nc.sync.dma_start(out=bias[nm], in_=d[nm][:, :])
            b3rep = singles.tile([128, 4 * 128], F32, tag="b3rep")
            nc.sync.dma_start(out=b3rep, in_=d["b3rep"][:, :])
            iota = singles.tile([128, WINDOW], F16, tag="iota")
            nc.sync.dma_start(out=iota, in_=d["iota"][:, :])
            lidxT = singles.tile([128, NSUB * nchunk], F32, tag="lidxT")
            nc.sync.dma_start(out=lidxT, in_=d["lidxT"][:, :])
            sidx_sb = singles.tile([128, nchunk, CHUNK_E // 16], I16, tag="sidx")
            didx_sb = singles.tile([128, nchunk, CHUNK_E // 16], I16, tag="didx")
            for r in range(8):
                nc.sync.dma_start(
                    out=sidx_sb[r * 16:(r + 1) * 16],
                    in_=d["sidxw"].rearrange("c p e -> p c e"),
                )
                nc.sync.dma_start(
                    out=didx_sb[r * 16:(r + 1) * 16],
                    in_=d["didxw"].rearrange("c p e -> p c e"),
                )
            # bias tile for Rsqrt(4*(var+eps)) = 0.5/sqrt(var+eps)
            eps4 = singles.tile([128, 1], F32, tag="eps4")
            nc.vector.memset(eps4, 4.0 * LN_EPS)

            import os as _os2
            skip_compute = int(_os2.environ.get("KERNEL_SKIP_COMPUTE", "0"))
            skip_gather = int(_os2.environ.get("KERNEL_SKIP_GATHER", "0"))
            no_act = int(_os2.environ.get("KERNEL_NO_ACT", "0"))
            no_bn = int(_os2.environ.get("KERNEL_NO_BN", "0"))
            no_ts = int(_os2.environ.get("KERNEL_NO_TS", "0"))
            AF_G = AF.Gelu
            AF_T = AF.Gelu if no_act else AF.Tanh
            AF_S = AF.Gelu if no_act else AF.Sqrt
            for _rep in range(reps):
              for c in range(nchunk):
                fsT = gath.tile([128, 1, CHUNK_E], F16, tag="fsT")
                for b in range(NBUCK) if not skip_gather else []:
                    nc.gpsimd.dma_gather(
                        out_ap=fsT[:, :, b * RUN:(b + 1) * RUN],
                        in_ap=ftab_rows[b * browse:(b + 1) * browse, :],
                        idxs_ap=sidx_sb[:, c, b * (RUN // 16):
                                        (b + 1) * (RUN // 16)],
                        num_idxs=RUN,
                        num_idxs_reg=RUN,
                        elem_size=H,
                        transpose=True,
                        single_packet=False,
                        queue_num=0,
                    )
                fdT = gath.tile([128, 1, CHUNK_E], F16, tag="fdT")
                if skip_gather:
                    nc.vector.memset(fdT[:, :, 0:2], 0)
                    nc.vector.memset(fsT[:, :, 0:2], 0)
                else:
                  nc.gpsimd.dma_gather(
                    out_ap=fdT,
                    in_ap=d["fshard"][:, :],
                    idxs_ap=didx_sb[:, c, :],
                    num_idxs=CHUNK_E,
                    num_idxs_reg=CHUNK_E,
                    elem_size=H,
                    transpose=True,
                    single_packet=False,
                    queue_num=0,
                  )
                if skip_compute:
                    z16 = outp.tile([128, 4, 128], F16, tag="osb")
                    nc.vector.tensor_copy(z16, fsT.rearrange(
                        "p a (q f) -> p (a q) f", q=4)[:, :, 0:128])
                    nc.sync.dma_start(
                        out=staging[c].rearrange("hh j d -> j hh d"),
                        in_=z16,
                    )
                    continue
                # -- phase A: edge MLP + LN stats per 512-quarter; gate
                #    matmuls accumulate into one chunk-wide PSUM tile --
                gatep = psmall.tile([128, NSUB], F32, tag="gatep", bufs=2)
                mv = lnp.tile([128, NSUB, 2], F32, tag="mv")
                xs = []
                for gi in range(4):
                    e0 = gi * 512
                    fs = fsT[:, 0, e0:e0 + 512]
                    fd = fdT[:, 0, e0:e0 + 512]

                    h1p = ppack.tile([128, 512], F32, tag="big")
                    nc.tensor.matmul(h1p, w["A1s"], fs, start=True, stop=False)
                    nc.tensor.matmul(h1p, w["A1d"], fd, start=False, stop=True)
                    g1p = ppack.tile([128, 512], F32, tag="big")
                    nc.tensor.matmul(g1p, w["Ag1s"], fs, start=True, stop=False)
                    nc.tensor.matmul(g1p, w["Ag1d"], fd, start=False, stop=True)

                    h1s = acts.tile([128, 512], F16, tag="h1s")
                    nc.scalar.activation(h1s, h1p, AF_G, bias=bias["b1f"])
                    h2p = ppack.tile([128, 512], F32, tag="big")
                    nc.tensor.matmul(h2p, w["W2"], h1s, start=True, stop=True)
                    h2s = acts.tile([128, 512], F16, tag="h2s")
                    nc.scalar.activation(h2s, h2p, AF_G, bias=bias["b2"])
                    g1s = acts.tile([128, 512], F16, tag="g1s")
                    nc.scalar.activation(g1s, g1p, AF_G, bias=bias["bg1f"])

                    # msg_pre (un-transposed, [edge, feat]) and gate pre-act
                    msgp = ppack.tile([128, 512], F32, tag="big")
                    for s in range(4):
                        sl = slice(s * 128, (s + 1) * 128)
                        nc.tensor.matmul(
                            msgp[:, sl], h2s[:, sl], w["W3"],
                            start=True, stop=True, skip_group_check=True,
                        )
                        k = gi * 4 + s
                        nc.tensor.matmul(
                            gatep[:, k:k + 1], g1s[:, sl], w["Wg2"],
                            start=True, stop=True, skip_group_check=True,
                        )
                    # x = msg_pre + b3 (fp16), then per-subtile LN stats
                    x = lnp.tile([128, 4, 128], F16, tag="x", bufs=8)
                    nc.vector.tensor_tensor(
                        x, msgp.rearrange("p (s f) -> p s f", s=4),
                        b3rep.rearrange("p (s f) -> p s f", s=4),
                        op=ALU.add,
                    )
                    xs.append(x)
                    if no_bn:
                        nc.vector.memset(mv[:, gi * 4:(gi + 1) * 4, :], 0.25)
                    else:
                        st = lnp.tile([128, 4, 6], F32, tag="st")
                        for s in range(4):
                            k = gi * 4 + s
                            nc.vector.bn_stats(st[:, s, :], x[:, s, :])
                            nc.vector.bn_aggr(mv[:, k, :], st[:, s, :])

                # -- phase B (chunk-wide): gate = 0.5*(tanh+1) and
                #    0.5*rstd = Rsqrt(4*(var+eps)); sc = their product.
                #    Scalar/vector engines only — gpsimd is reserved for
                #    dma_gather descriptor generation --
                NG = NSUB
                gate_t = lnp.tile([128, NG], F32, tag="gate_t")
                nc.scalar.activation(gate_t, gatep, AF_T,
                                     bias=bias["bg2"], scale=0.5)
                g2t = lnp.tile([128, NG], F32, tag="g2t")
                nc.vector.tensor_scalar(g2t, gate_t, 1.0, None, op0=ALU.add)
                std2 = lnp.tile([128, NG], F32, tag="std2")
                nc.scalar.activation(std2, mv[:, :, 1], AF_S,
                                     bias=eps4, scale=4.0)
                rstd_h = lnp.tile([128, NG], F32, tag="rstd_h")
                nc.vector.reciprocal(rstd_h, std2)
                sc = lnp.tile([128, NG], F32, tag="sc")
                nc.vector.tensor_tensor(sc, rstd_h, g2t, op=ALU.mult)

                # -- phase C: center, one-hot (scaled), segment-sum into the
                #    chunk-wide 512 window, then W_out --
                updp = pwin.tile([128, WINDOW], F32, tag="win")
                for gi in range(4):
                    x = xs[gi]
                    msg16 = acts.tile([128, 512], F16, tag="msg16")
                    A = acts.tile([128, 4, WINDOW], F16, tag="A", bufs=4)
                    if no_ts:
                        nc.vector.tensor_copy(msg16, x.rearrange("p s f -> p (s f)"))
                        nc.vector.memset(A, 0.001)
                    else:
                      for s in range(4):
                        sl = slice(s * 128, (s + 1) * 128)
                        k = gi * 4 + s
                        nc.vector.tensor_scalar(
                            msg16[:, sl], x[:, s, :],
                            mv[:, k, 0:1], None, op0=ALU.subtract,
                        )
                        nc.vector.tensor_scalar(
                            A[:, s, :], iota,
                            lidxT[:, c * NSUB + k: c * NSUB + k + 1],
                            sc[:, k:k + 1],
                            op0=ALU.is_equal, op1=ALU.mult,
                        )
                    for s in range(4):
                        sl = slice(s * 128, (s + 1) * 128)
                        k = gi * 4 + s
                        nc.tensor.matmul(
                            updp, msg16[:, sl], A[:, s, :],
                            start=(k == 0), stop=(k == NSUB - 1),
                            skip_group_check=True,
                        )
                upd16 = outp.tile([128, WINDOW], F16, tag="upd16")
                if c % 2 == 0:
                    nc.vector.tensor_copy(upd16, updp)
                else:
                    nc.scalar.activation(upd16, updp, AF.Copy)

                o4 = psmall.tile([128, 4, 128], F32, tag="o4", bufs=1)
                for hh in range(4):
                    nc.tensor.matmul(
                        o4[:, hh, :], upd16[:, hh * 128:(hh + 1) * 128],
                        w["W_out"], start=True, stop=True,
                        skip_group_check=True,
                    )
                osb = outp.tile([128, 4, 128], F16, tag="osb")
                if c % 2 == 0:
                    nc.scalar.activation(osb, o4, AF.Copy)
                else:
                    nc.vector.tensor_copy(osb, o4)
                nc.sync.dma_start(
                    out=staging[c].rearrange("hh j d -> j hh d"),
                    in_=osb,
                )
    nc.finalize()
    return nc


# --------------------------------------------------------------------------
# entry point
# --------------------------------------------------------------------------

_LAST_PERF = {}


def kernel(**inputs):
    import os
    import time as _time
    prep = _prepare(inputs)
    reps = int(os.environ.get("KERNEL_REPS", "1"))
    nc = _build(prep["nchunk"], prep["npc_pad"], prep["browse"], reps=reps)
    trace = bool(int(os.environ.get("KERNEL_TRACE", "0")))
    res = run_bass_kernel_spmd(
        nc, prep["in_maps"], core_ids=list(range(N_CORES)), trace=trace,
    )
    nrep = int(os.environ.get("KERNEL_REPEAT", "0"))
    if nrep:
        walls = []
        for _ in range(nrep):
            t0 = _time.time()
            res = run_bass_kernel_spmd(
                nc, prep["in_maps"], core_ids=list(range(N_CORES)), trace=trace,
            )
            walls.append(_time.time() - t0)
        _rw = min(walls)
        print("repeat walls (ms):", " ".join("%.0f" % (w * 1e3) for w in walls))
    else:
        _rw = None
    _LAST_PERF.clear()
    _LAST_PERF.update(
        repeat_wall_s=_rw,
        exec_time_ns=res.exec_time_ns,
        mean_exec_time_ns=res.mean_exec_time_ns,
        trace=res.instructions_and_trace[1] if res.instructions_and_trace else None,
    )

    N = prep["N"]
    out = np.zeros((N + WINDOW, H), np.float64)
    for c in range(N_CORES):
        stg = res.results[c]["staging"].reshape(prep["nchunk"], WINDOW, H)
        bases = prep["meta"][c]
        for g in range(prep["nchunk"]):
            b = int(bases[g])
            out[b: b + WINDOW] += stg[g]
    out = out[:N] + prep["b_out"]
    return out.astype(np.float32)


# revision 9
# speedup vs baseline: 1.4059x; 1.1333x over previous
"""Edge-parallel GNN message-passing kernel for 8 Trainium2 NeuronCores.

v4: node features are SHARDED across the 8 cores (3.2MB each instead of a
15MB replicated gather table); the full table is assembled on-device with an
AllGather collective, and the per-edge source gathers read it through four
bucketed sub-gathers (int16 gather indices only span 25088 rows, so each
chunk's 2048 slots are laid out as four fixed 512-slot bucket runs).
Destination gathers read the core's own shard directly.  WINDOW=512 with one
2048-edge group per chunk; each chunk's gated messages are segment-summed
into a [128, 512] PSUM window and a dense [512, 128] staging tile is
written.

  * Host: sort edges by destination, split nodes into 8 contiguous ranges;
    each core gets every edge whose destination is in its range.
  * Node-level src/dst transforms folded into the first edge MLP layer on
    the host (linear, no activation in between):
      h1_pre = fsrc @ (W_src@W1a) + fdst @ (W_dst@W1b) + b1f
      g1_pre = fsrc @ (W_src@Wg1a) + fdst @ (W_dst@Wg1b) + bg1f
  * Device, per 2048-edge chunk: gather fp16 feature rows transposed
    ([feat, edge] layout) with dma_gather, run the MLP/gate/LayerNorm with
    fp32 PSUM accumulation, segment-sum into a 512-node window via one-hot
    matmuls, apply W_out, write a dense [512, 128] staging tile.
  * Host: accumulate the (overlapping) staging windows into [N, DOUT].
"""

import sys

sys.path.insert(0, "/opt/trn_rl_repo")

import numpy as np

import concourse.bass as bass
import concourse.bacc as bacc
import concourse.tile as tile
from concourse import mybir
from concourse.bass_utils import run_bass_kernel_spmd

N_CORES = 8
H = 128
WINDOW = 512          # destination-node window per chunk (PSUM free dim)
CHUNK_E = 2048        # edge slots per chunk
NSUB = CHUNK_E // 128  # 16 subtiles
NBUCK = 4             # gather-index buckets (int16 limit / table rows)
RUN = CHUNK_E // NBUCK  # slots per bucket run (fixed, SPMD-uniform)
LN_EPS = 1e-5
F16 = mybir.dt.float16
F32 = mybir.dt.float32
I16 = mybir.dt.int16
I32 = mybir.dt.int32
AF = mybir.ActivationFunctionType
ALU = mybir.AluOpType


# --------------------------------------------------------------------------
# host-side packing
# --------------------------------------------------------------------------

def _pack_core(ed_c, buck_c):
    """Split one core's dst-sorted edges into chunks whose destinations span
    <WINDOW nodes and which have <=RUN edges in each source bucket.
    Returns per-chunk (start, end, base)."""
    out = []
    i = 0
    n = len(ed_c)
    # prefix counts per bucket for O(1) range counts
    pref = np.zeros((NBUCK, n + 1), np.int64)
    for b in range(NBUCK):
        pref[b, 1:] = np.cumsum(buck_c == b)
    while i < n:
        base = int(ed_c[i])
        j = min(
            i + CHUNK_E,
            int(np.searchsorted(ed_c, base + WINDOW, side="left")),
        )
        # shrink j until every bucket count fits in RUN
        while True:
            over = 0
            for b in range(NBUCK):
                cnt = int(pref[b, j] - pref[b, i])
                if cnt > RUN:
                    over = max(over, cnt - RUN)
            if over == 0:
                break
            j -= over
        out.append((i, j, base))
        i = j
    return out


def _prepare(inputs):
    feat = np.ascontiguousarray(np.asarray(inputs["feat"], np.float32))
    es = np.asarray(inputs["edge_src"]).astype(np.int64)
    ed = np.asarray(inputs["edge_dst"]).astype(np.int64)
    N, DIN = feat.shape
    E = es.shape[0]
    npc = -(-N // N_CORES)  # nodes per core (ceil)

    f64 = np.float64
    W_src = np.asarray(inputs["W_src"], f64)
    W_dst = np.asarray(inputs["W_dst"], f64)
    W1a = np.asarray(inputs["W1a"], f64)
    W1b = np.asarray(inputs["W1b"], f64)
    Wg1a = np.asarray(inputs["Wg1a"], f64)
    Wg1b = np.asarray(inputs["Wg1b"], f64)
    b_src = np.asarray(inputs["b_src"], f64)
    b_dst = np.asarray(inputs["b_dst"], f64)
    ln_g = np.asarray(inputs["ln_g"], f64)
    ln_b = np.asarray(inputs["ln_b"], f64)
    if not np.allclose(ln_b, 0.0):
        raise NotImplementedError("non-zero ln_b not supported")

    wpack = {
        "A1s": W_src @ W1a,
        "A1d": W_dst @ W1b,
        "Ag1s": W_src @ Wg1a,
        "Ag1d": W_dst @ Wg1b,
        "W2": np.asarray(inputs["W2"], f64),
        "W3": np.asarray(inputs["W3"], f64),
        "W_out": np.diag(ln_g) @ np.asarray(inputs["W_out"], f64),
    }
    b1f = np.asarray(inputs["b1"], f64) + b_src @ W1a + b_dst @ W1b
    bg1f = np.asarray(inputs["bg1"], f64) + b_src @ Wg1a + b_dst @ Wg1b
    Wg2 = np.asarray(inputs["Wg2"], f64)  # [H, 1]
    bg2 = float(np.asarray(inputs["bg2"], f64).reshape(()))

    feat16 = feat.astype(np.float16)
    npc_pad = -(-npc // 128) * 128       # padded shard rows
    browse = 2 * npc_pad                 # rows per gather bucket (int16-safe)

    # global source node id -> (bucket, in-bucket table row)
    def node_to_row(nid):
        return (nid // npc) * npc_pad + (nid % npc)

    order = np.argsort(ed, kind="stable")
    es_s = es[order]
    ed_s = ed[order]
    src_rows = node_to_row(es_s)
    src_buck = src_rows // browse
    bounds = np.searchsorted(ed_s, np.arange(N_CORES + 1) * npc, side="left")

    core_chunks = []
    for c in range(N_CORES):
        lo, hi = int(bounds[c]), int(bounds[c + 1])
        core_chunks.append(_pack_core(ed_s[lo:hi], src_buck[lo:hi]))

    nchunk = max(len(g) for g in core_chunks)

    # device input arrays per core
    in_maps = []
    meta = []
    for c in range(N_CORES):
        lo = int(bounds[c])
        chunks = core_chunks[c]
        sidx = np.zeros((nchunk, CHUNK_E), np.int64)
        didx = np.zeros((nchunk, CHUNK_E), np.int64)
        lidx = np.full((nchunk, CHUNK_E), -1.0, np.float32)
        bases = np.zeros(nchunk, np.int64)
        c0 = c * npc
        for g, (i, j, base) in enumerate(chunks):
            bases[g] = base
            eb = src_buck[lo + i: lo + j]
            erow = src_rows[lo + i: lo + j] % browse
            edst = ed_s[lo + i: lo + j] - base
            eloc = ed_s[lo + i: lo + j] - c0
            for b in range(NBUCK):
                m = eb == b
                k = int(m.sum())
                assert k <= RUN
                s0 = b * RUN
                sidx[g, s0: s0 + k] = erow[m]
                didx[g, s0: s0 + k] = eloc[m]
                lidx[g, s0: s0 + k] = edst[m].astype(np.float32)

        fshard = np.zeros((npc_pad, H), np.float16)
        seg = feat16[c0: min(c0 + npc, N)]
        fshard[: len(seg)] = seg

        def wrap16(idx):  # [nchunk, CHUNK_E] -> [nchunk, 16, CHUNK_E//16]
            return np.ascontiguousarray(
                idx.reshape(nchunk, CHUNK_E // 16, 16).transpose(0, 2, 1)
            ).astype(np.int16)

        # lidx transposed for per-partition access: [128, NSUB*nchunk]
        lidxT = np.ascontiguousarray(
            lidx.reshape(nchunk, NSUB, 128).transpose(2, 0, 1).reshape(128, -1)
        ).astype(np.float32)

        im = {
            "fshard": fshard,
            "sidxw": wrap16(sidx),
            "didxw": wrap16(didx),
            "lidxT": lidxT,
            "b3rep": np.tile(np.asarray(inputs["b3"], np.float16), (128, 4)),
            "b1f": b1f.astype(np.float32).reshape(H, 1),
            "bg1f": bg1f.astype(np.float32).reshape(H, 1),
            "b2": np.asarray(inputs["b2"], np.float32).reshape(H, 1),
            "bg2": np.full((128, 1), 0.5 * bg2, np.float32),  # tanh-form sigmoid
            "Wg2": Wg2.astype(np.float16),
        }
        for k, v in wpack.items():
            im[k] = v.astype(np.float16)
        in_maps.append(im)
        meta.append(bases)

    b_out = np.asarray(inputs["b_out"], np.float64)
    return dict(
        in_maps=in_maps, meta=meta, nchunk=nchunk, npc_pad=npc_pad,
        browse=browse, N=N, b_out=b_out,
    )


# --------------------------------------------------------------------------
# device kernel builder
# --------------------------------------------------------------------------

def _build(nchunk, npc_pad, browse, reps=1):
    nc = bacc.Bacc("TRN2", target_bir_lowering=False, debug=False,
                   num_devices=N_CORES)
    d = {}
    d["fshard"] = nc.dram_tensor("fshard", [npc_pad, H], F16,
                                 kind="ExternalInput")
    d["sidxw"] = nc.dram_tensor("sidxw", [nchunk, 16, CHUNK_E // 16], I16,
                                kind="ExternalInput")
    d["didxw"] = nc.dram_tensor("didxw", [nchunk, 16, CHUNK_E // 16], I16,
                                kind="ExternalInput")
    d["lidxT"] = nc.dram_tensor("lidxT", [128, NSUB * nchunk], F32,
                                kind="ExternalInput")
    d["b3rep"] = nc.dram_tensor("b3rep", [128, 4 * 128], F16,
                                kind="ExternalInput")
    for nm in ("b1f", "bg1f", "b2", "bg2"):
        d[nm] = nc.dram_tensor(nm, [128, 1], F32, kind="ExternalInput")
    for nm in ("A1s", "A1d", "Ag1s", "Ag1d", "W2", "W3", "W_out"):
        d[nm] = nc.dram_tensor(nm, [H, H], F16, kind="ExternalInput")
    d["Wg2"] = nc.dram_tensor("Wg2", [H, 1], F16, kind="ExternalInput")
    staging = nc.dram_tensor("staging", [nchunk, 4, 128, 128], F16,
                             kind="ExternalOutput")

    with tile.TileContext(nc) as tc:
        with (
            tc.tile_pool(name="dram", bufs=1, space="DRAM") as dram,
            tc.tile_pool(name="singles", bufs=1) as singles,
            tc.tile_pool(name="gath", bufs=3) as gath,
            tc.tile_pool(name="acts", bufs=3) as acts,
            tc.tile_pool(name="ln", bufs=3) as lnp,
            tc.tile_pool(name="outp", bufs=3) as outp,
            tc.tile_pool(name="ppack", bufs=3, space="PSUM") as ppack,
            tc.tile_pool(name="pwin", bufs=2, space="PSUM") as pwin,
            tc.tile_pool(name="psmall", bufs=2, space="PSUM") as psmall,
        ):
            # ---- preamble: AllGather the feature shards into a full
            #      on-device table (collectives cannot read IO tensors,
            #      so bounce through an internal DRAM tile) ----
            bounce = dram.tile([npc_pad, H], F16, tag="bounce")
            nc.sync.dma_start(out=bounce[:], in_=d["fshard"][:, :])
            ftab = dram.tile([N_CORES, npc_pad, H], F16, tag="ftab",
                             addr_space="Shared")
            import os as _os
            if int(_os.environ.get("KERNEL_NO_COLL", "0")):
                nc.sync.dma_start(out=ftab[0], in_=bounce[:])
            else:
                nc.gpsimd.collective_compute(
                    "AllGather",
                    mybir.AluOpType.bypass,
                    replica_groups=[list(range(N_CORES))],
                    ins=[bounce[:]],
                    outs=[ftab[:]],
                )
            ftab_rows = ftab[:].rearrange("c r f -> (c r) f")

            # ---- preamble: constants into SBUF ----
            w = {}
            for nm in ("A1s", "A1d", "Ag1s", "Ag1d", "W2", "W3", "W_out"):
                w[nm] = singles.tile([H, H], F16, tag=nm, name=nm)
                nc.sync.dma_start(out=w[nm], in_=d[nm][:, :])
            w["Wg2"] = singles.tile([H, 1], F16, tag="Wg2", name="Wg2")
            nc.sync.dma_start(out=w["Wg2"], in_=d["Wg2"][:, :])
            bias = {}
            for nm in ("b1f", "bg1f", "b2", "bg2"):
                bias[nm] = singles.tile([128, 1], F32, tag=nm, name=nm)
                nc.sync.dma_start(out=bias[nm], in_=d[nm][:, :])
            b3rep = singles.tile([128, 4 * 128], F16, tag="b3rep")
            nc.sync.dma_start(out=b3rep, in_=d["b3rep"][:, :])
            iota_i = singles.tile([128, WINDOW], I16, tag="iota_i")
            nc.gpsimd.iota(iota_i, pattern=[[1, WINDOW]], base=0,
                           channel_multiplier=0)
            iota = singles.tile([128, WINDOW], F16, tag="iota")
            nc.vector.tensor_copy(iota, iota_i)
            lidxT = singles.tile([128, NSUB * nchunk], F32, tag="lidxT")
            nc.sync.dma_start(out=lidxT, in_=d["lidxT"][:, :])
            sidx_sb = singles.tile([128, nchunk, CHUNK_E // 16], I16, tag="sidx")
            didx_sb = singles.tile([128, nchunk, CHUNK_E // 16], I16, tag="didx")
            for r in range(8):
                nc.sync.dma_start(
                    out=sidx_sb[r * 16:(r + 1) * 16],
                    in_=d["sidxw"].rearrange("c p e -> p c e"),
                )
                nc.sync.dma_start(
                    out=didx_sb[r * 16:(r + 1) * 16],
                    in_=d["didxw"].rearrange("c p e -> p c e"),
                )
            # bias tile for Rsqrt(4*(var+eps)) = 0.5/sqrt(var+eps)
            eps4 = singles.tile([128, 1], F32, tag="eps4")
            nc.vector.memset(eps4, 4.0 * LN_EPS)

            import os as _os2
            skip_compute = int(_os2.environ.get("KERNEL_SKIP_COMPUTE", "0"))
            skip_gather = int(_os2.environ.get("KERNEL_SKIP_GATHER", "0"))
            no_act = int(_os2.environ.get("KERNEL_NO_ACT", "0"))
            no_bn = int(_os2.environ.get("KERNEL_NO_BN", "0"))
            no_ts = int(_os2.environ.get("KERNEL_NO_TS", "0"))
            AF_G = AF.Gelu
            AF_T = AF.Gelu if no_act else AF.Tanh
            AF_S = AF.Gelu if no_act else AF.Sqrt
            for _rep in range(reps):
              for c in range(nchunk):
                fsT = gath.tile([128, 1, CHUNK_E], F16, tag="fsT")
                for b in range(NBUCK) if not skip_gather else []:
                    nc.gpsimd.dma_gather(
                        out_ap=fsT[:, :, b * RUN:(b + 1) * RUN],
                        in_ap=ftab_rows[b * browse:(b + 1) * browse, :],
                        idxs_ap=sidx_sb[:, c, b * (RUN // 16):
                                        (b + 1) * (RUN // 16)],
                        num_idxs=RUN,
                        num_idxs_reg=RUN,
                        elem_size=H,
                        transpose=True,
                        single_packet=False,
                        queue_num=0,
                    )
                fdT = gath.tile([128, 1, CHUNK_E], F16, tag="fdT")
                if skip_gather:
                    nc.vector.memset(fdT[:, :, 0:2], 0)
                    nc.vector.memset(fsT[:, :, 0:2], 0)
                else:
                  nc.gpsimd.dma_gather(
                    out_ap=fdT,
                    in_ap=d["fshard"][:, :],
                    idxs_ap=didx_sb[:, c, :],
                    num_idxs=CHUNK_E,
                    num_idxs_reg=CHUNK_E,
                    elem_size=H,
                    transpose=True,
                    single_packet=False,
                    queue_num=0,
                  )
                if skip_compute:
                    z16 = outp.tile([128, 4, 128], F16, tag="osb")
                    nc.vector.tensor_copy(z16, fsT.rearrange(
                        "p a (q f) -> p (a q) f", q=4)[:, :, 0:128])
                    nc.sync.dma_start(
                        out=staging[c].rearrange("hh j d -> j hh d"),
                        in_=z16,
                    )
                    continue
                # -- phase A: edge MLP + LN stats per 512-quarter; gate
                #    matmuls accumulate into one chunk-wide PSUM tile --
                gatep = psmall.tile([128, NSUB], F32, tag="gatep", bufs=2)
                mv = lnp.tile([128, NSUB, 2], F32, tag="mv")
                xs = []
                for gi in range(4):
                    e0 = gi * 512
                    fs = fsT[:, 0, e0:e0 + 512]
                    fd = fdT[:, 0, e0:e0 + 512]

                    h1p = ppack.tile([128, 512], F32, tag="big")
                    nc.tensor.matmul(h1p, w["A1s"], fs, start=True, stop=False)
                    nc.tensor.matmul(h1p, w["A1d"], fd, start=False, stop=True)
                    g1p = ppack.tile([128, 512], F32, tag="big")
                    nc.tensor.matmul(g1p, w["Ag1s"], fs, start=True, stop=False)
                    nc.tensor.matmul(g1p, w["Ag1d"], fd, start=False, stop=True)

                    h1s = acts.tile([128, 512], F16, tag="h1s")
                    nc.scalar.activation(h1s, h1p, AF_G, bias=bias["b1f"])
                    h2p = ppack.tile([128, 512], F32, tag="big")
                    nc.tensor.matmul(h2p, w["W2"], h1s, start=True, stop=True)
                    h2s = acts.tile([128, 512], F16, tag="h2s")
                    nc.scalar.activation(h2s, h2p, AF_G, bias=bias["b2"])
                    g1s = acts.tile([128, 512], F16, tag="g1s")
                    nc.scalar.activation(g1s, g1p, AF_G, bias=bias["bg1f"])

                    # msg_pre (un-transposed, [edge, feat]) and gate pre-act
                    msgp = ppack.tile([128, 512], F32, tag="big")
                    for s in range(4):
                        sl = slice(s * 128, (s + 1) * 128)
                        nc.tensor.matmul(
                            msgp[:, sl], h2s[:, sl], w["W3"],
                            start=True, stop=True, skip_group_check=True,
                        )
                        k = gi * 4 + s
                        nc.tensor.matmul(
                            gatep[:, k:k + 1], g1s[:, sl], w["Wg2"],
                            start=True, stop=True, skip_group_check=True,
                        )
                    # x = msg_pre + b3 (fp16), then per-subtile LN stats
                    x = lnp.tile([128, 4, 128], F16, tag="x", bufs=8)
                    nc.vector.tensor_tensor(
                        x, msgp.rearrange("p (s f) -> p s f", s=4),
                        b3rep.rearrange("p (s f) -> p s f", s=4),
                        op=ALU.add,
                    )
                    xs.append(x)
                    if no_bn:
                        nc.vector.memset(mv[:, gi * 4:(gi + 1) * 4, :], 0.25)
                    else:
                        st = lnp.tile([128, 4, 6], F32, tag="st")
                        for s in range(4):
                            k = gi * 4 + s
                            nc.vector.bn_stats(st[:, s, :], x[:, s, :])
                            nc.vector.bn_aggr(mv[:, k, :], st[:, s, :])

                # -- phase B (chunk-wide): gate = 0.5*(tanh+1) and
                #    0.5*rstd = Rsqrt(4*(var+eps)); sc = their product.
                #    Scalar/vector engines only — gpsimd is reserved for
                #    dma_gather descriptor generation --
                NG = NSUB
                gate_t = lnp.tile([128, NG], F32, tag="gate_t")
                nc.scalar.activation(gate_t, gatep, AF_T,
                                     bias=bias["bg2"], scale=0.5)
                g2t = lnp.tile([128, NG], F32, tag="g2t")
                nc.vector.tensor_scalar(g2t, gate_t, 1.0, None, op0=ALU.add)
                std2 = lnp.tile([128, NG], F32, tag="std2")
                nc.scalar.activation(std2, mv[:, :, 1], AF_S,
                                     bias=eps4, scale=4.0)
                rstd_h = lnp.tile([128, NG], F32, tag="rstd_h")
                nc.vector.reciprocal(rstd_h, std2)
                sc = lnp.tile([128, NG], F32, tag="sc")
                nc.vector.tensor_tensor(sc, rstd_h, g2t, op=ALU.mult)

                # -- phase C: center, one-hot (scaled), segment-sum into the
                #    chunk-wide 512 window, then W_out --
                updp = pwin.tile([128, WINDOW], F32, tag="win")
                for gi in range(4):
                    x = xs[gi]
                    msg16 = acts.tile([128, 512], F16, tag="msg16")
                    A = acts.tile([128, 4, WINDOW], F16, tag="A", bufs=4)
                    if no_ts:
                        nc.vector.tensor_copy(msg16, x.rearrange("p s f -> p (s f)"))
                        nc.vector.memset(A, 0.001)
                    else:
                      for s in range(4):
                        sl = slice(s * 128, (s + 1) * 128)
                        k = gi * 4 + s
                        nc.vector.tensor_scalar(
                            msg16[:, sl], x[:, s, :],
                            mv[:, k, 0:1], None, op0=ALU.subtract,
                        )
                        nc.vector.tensor_scalar(
                            A[:, s, :], iota,
                            lidxT[:, c * NSUB + k: c * NSUB + k + 1],
                            sc[:, k:k + 1],
                            op0=ALU.is_equal, op1=ALU.mult,
                        )
                    for s in range(4):
                        sl = slice(s * 128, (s + 1) * 128)
                        k = gi * 4 + s
                        nc.tensor.matmul(
                            updp, msg16[:, sl], A[:, s, :],
                            start=(k == 0), stop=(k == NSUB - 1),
                            skip_group_check=True,
                        )
                upd16 = outp.tile([128, WINDOW], F16, tag="upd16")
                if c % 2 == 0:
                    nc.vector.tensor_copy(upd16, updp)
                else:
                    nc.scalar.activation(upd16, updp, AF.Copy)

                o4 = psmall.tile([128, 4, 128], F32, tag="o4", bufs=1)
                for hh in range(4):
                    nc.tensor.matmul(
                        o4[:, hh, :], upd16[:, hh * 128:(hh + 1) * 128],
                        w["W_out"], start=True, stop=True,
                        skip_group_check=True,
                    )
                osb = outp.tile([128, 4, 128], F16, tag="osb")
                if c % 2 == 0:
                    nc.scalar.activation(osb, o4, AF.Copy)
                else:
                    nc.vector.tensor_copy(osb, o4)
                nc.sync.dma_start(
                    out=staging[c].rearrange("hh j d -> j hh d"),
                    in_=osb,
                )
    nc.finalize()
    return nc


# --------------------------------------------------------------------------
# entry point
# --------------------------------------------------------------------------

_LAST_PERF = {}


def kernel(**inputs):
    import os
    import time as _time
    prep = _prepare(inputs)
    reps = int(os.environ.get("KERNEL_REPS", "1"))
    nc = _build(prep["nchunk"], prep["npc_pad"], prep["browse"], reps=reps)
    trace = bool(int(os.environ.get("KERNEL_TRACE", "0")))
    res = run_bass_kernel_spmd(
        nc, prep["in_maps"], core_ids=list(range(N_CORES)), trace=trace,
    )
    nrep = int(os.environ.get("KERNEL_REPEAT", "0"))
    if nrep:
        walls = []
        for _ in range(nrep):
            t0 = _time.time()
            res = run_bass_kernel_spmd(
                nc, prep["in_maps"], core_ids=list(range(N_CORES)), trace=trace,
            )
            walls.append(_time.time() - t0)
        _rw = min(walls)
        print("repeat walls (ms):", " ".join("%.0f" % (w * 1e3) for w in walls))
    else:
        _rw = None
    _LAST_PERF.clear()
    _LAST_PERF.update(
        repeat_wall_s=_rw,
        exec_time_ns=res.exec_time_ns,
        mean_exec_time_ns=res.mean_exec_time_ns,
        trace=res.instructions_and_trace[1] if res.instructions_and_trace else None,
    )

    N = prep["N"]
    out = np.zeros((N + WINDOW, H), np.float64)
    for c in range(N_CORES):
        stg = res.results[c]["staging"].reshape(prep["nchunk"], WINDOW, H)
        bases = prep["meta"][c]
        for g in range(prep["nchunk"]):
            b = int(bases[g])
            out[b: b + WINDOW] += stg[g]
    out = out[:N] + prep["b_out"]
    return out.astype(np.float32)
